# revision 12
# baseline (speedup 1.0000x reference)
"""MemAELoss (MSE + entropy regularizer + pairwise-cosine memory penalty) on 8 trn2 cores.

Math (validated vs reference):
  loss = mean((g-o)^2) - 2e-4 * sum(softmax(att)*log_softmax(att))
         + sum_{i<j} cos(mem_i, mem_j)

Reformulations:
  * entropy per row, no max-subtraction needed (|att| < 6): S1 = sum e^x,
    S2 = sum x*e^x, row_term = S2/S1 - ln(S1). Per-row S1/S2 are exported
    and the tiny ln-finalize (8K rows) runs on the host during the gather,
    which keeps the ACT table set fixed (exp only) on device.
  * cosine triu sum: with u_i = mem_i/||mem_i||,
      sum_{i<j} u_i.u_j = 0.5*(||sum_i u_i||^2 - sum_i ||u_i||^2)
    so each core only produces a 256-vector s_c and a scalar d_c. The row
    normalization (u = mem/||mem||) happens on the host during input
    marshaling; the device computes the O(N^2)-equivalent reduction
    (s = ones^T @ u on PE, d = sum u^2 via DVE accum).

Sharding: pure data-parallel across 8 cores (output/ground_truth by flat
range, att by rows, mem by rows padded 250->256 with zero rows).

Performance structure (per core, ~16.5us DMA floor, ~16.3us ACT floor):
  * output/ground_truth ship as fp8 e3m4 (the loss is a statistical
    aggregate; e3m4 rounding perturbs it ~1.4e-5 rel, measured end-to-end,
    vs 2e-2 tolerance). att ships as f16: the x*e^x multiply must run on
    DVE in its 2x 16-bit mode (a 1-byte operand drops DVE to 1x), and no
    engine/DMA path upcasts fp8->f16 at line rate (the SWDGE cast DMA
    works but runs ~30x below line rate - measured 145us/iter). HBM per
    core: 5.93 MB vs 7.5 MB for the all-f16 version.
  * xg (packed [x_t | g_t] mse tiles) stays fp8 in SBUF: Pool subtracts
    fp8-fp8 -> f16 exactly; DVE squares the f16 diff at 2x with f32 accum.
  * engine placement: ACT does ONLY the 8 exps (f16 out + f32 S1 accum),
    ~2.0us each - the per-core compute bottleneck (~16.3us). DVE: 8 x*e
    STT at 2x (+S2 accum), 6 mse squares, 2 mem-d accums, PSUM copies
    (~13.5us). Pool: 6 mse diffs (~8.5us). PE: 3 tiny matmuls. SP: all
    HWDGE loads + 2 stores (FIFO per ring => transfers arrive in emission
    order at full bandwidth, no concurrent-DMA bandwidth splitting).
  * loads emitted in consumption order (att singles with xg/m interleaved,
    a7 last-but-early); compute emitted in data-arrival order with mse
    squares slotted into the DVE gaps between x*e ops. The mse/mem
    finalize + o store complete before the entropy stream ends; the rr
    (S1/S2) store right after the last x*e is the tail.
  * avoid tensor_tensor_reduce and DMA accum_op - both fault on this
    toolchain/HW. Multi-sem waits legalized by Bacc's event semaphores.
"""

import sys

sys.path.insert(0, "/opt/trn_rl_repo")

import ml_dtypes
import numpy as np

import concourse.bacc as bacc
import concourse.tile as tile
from concourse import mybir
from concourse.bass_utils import run_bass_kernel_spmd
from concourse.tile import add_dep_helper

F32 = mybir.dt.float32
F16 = mybir.dt.float16
F8 = mybir.dt.float8e3
FP8NP = ml_dtypes.float8_e3m4
Alu = mybir.AluOpType
Act = mybir.ActivationFunctionType

N_CORES = 8
MSE_N = 32 * 3 * 256 * 256  # 6291456 total elements
MSE_FREE = 6144             # per-core: 128 x 6144 (x and g each)
MSE_CH = 1024               # mse accum-chunk width (6 chunks)
ATT_TILES = 8               # per-core att rows 1024 = 8 x 128
ATT_F = 2000
MSE_TILING = [(0, 2), (2, 2), (4, 1), (5, 1)]          # (chunk0, nchunks)
MEM_ROWS = 250              # per-core mem rows, padded to 256 (2 x 128)
REG_PARAM = 2e-4
NP = 8                      # 6 mse ssd chunks, 2 d

_prog = None


def _build_program(loop_iters=None):
    nc = bacc.Bacc()
    # All big inputs are fp8 e3m4 on the wire. xg packs [x_t | g_t] blocks
    # so mse x/g slices are adjacent; a packs att row-groups so partition p
    # holds rows {p, 128+p, ...} concatenated (any [128, k*2000] slice is
    # one clean 2D DMA). m holds host-normalized unit rows (pad rows = 0).
    xg = nc.declare_dram_parameter("xg", [128, 2 * MSE_FREE], F8, isOutput=False)
    a = nc.declare_dram_parameter("a", [128, ATT_TILES * ATT_F], F16, isOutput=False)
    m = nc.declare_dram_parameter("m", [128, 512], F32, isOutput=False)
    o_out = nc.declare_dram_parameter("o", [1, NP + 256], F32, isOutput=True)
    rr_out = nc.declare_dram_parameter("rr", [128, 2 * ATT_TILES], F32, isOutput=True)

    with tile.TileContext(nc) as tc:
        with (
            tc.tile_pool(name="att_in", bufs=2) as apool,
            tc.tile_pool(name="att_exp", bufs=2) as epool,
            tc.tile_pool(name="mse_in", bufs=2) as xpool,
            tc.tile_pool(name="mse_diff", bufs=4) as dpool,
            tc.tile_pool(name="mem", bufs=2) as mpool,
            tc.tile_pool(name="stats", bufs=2) as spool,
            tc.tile_pool(name="psum", bufs=2, space="PSUM") as ppool,
        ):

          def body(_iv=None):
            s12 = spool.tile([128, 2 * ATT_TILES], F32, tag="s12")
            fin = spool.tile([128, NP], F32, tag="fin")
            ones = spool.tile([128, 1], F32, tag="ones")
            nc.vector.memset(ones[:, :], 1.0)

            at = apool.tile([128, ATT_TILES * ATT_F], F16, tag="at")
            et = epool.tile([128, ATT_TILES * ATT_F], F16, tag="et")
            xgt = xpool.tile([128, 2 * MSE_FREE], F8, tag="xg")
            mt = mpool.tile([128, 512], F32, tag="m")

            # --- loads. Measured DMA behavior here: HWDGE transfers run at
            # full rate but each dma_start pays ~2us of serialized
            # completion latency on its ring (15 DMAs/iter => 46us/iter);
            # SWDGE (gpsimd) moves data at only ~33 GB/s. So: merge the big
            # loads and split them across the two HWDGE rings (att on SP,
            # xg on ACT - one extra ACT dispatch ~0.7us), and put only the
            # tiny m load / o store on SWDGE, where Pool pays ~1us of
            # descriptor-gen and the slow transfer rides async off the
            # critical path. rr (the tail output) stays on SP. At steady
            # state (For_i, bufs=2) iteration k+1's loads overlap iteration
            # k's compute, so arrival granularity costs nothing.
            nc.sync.dma_start(at[:, :], a[:, :])
            nc.scalar.dma_start(xgt[:, :], xg[:, :])
            nc.gpsimd.dma_start(mt[:, :], m[:, :])

            # --- ACT: the 8 exps, in arrival order (the only ACT work) ---
            for t in range(ATT_TILES):
                sl = slice(t * ATT_F, (t + 1) * ATT_F)
                nc.scalar.activation(
                    et[:, sl], at[:, sl], Act.Exp,
                    accum_out=s12[:, t : t + 1],
                )

            # --- DVE/Pool, emitted in expected arrival order; mse squares
            # and mem accums slot into the DVE idle gaps between x*e ops ---
            def xe(t):
                sl = slice(t * ATT_F, (t + 1) * ATT_F)
                nc.vector.scalar_tensor_tensor(
                    et[:, sl], at[:, sl], 1.0, et[:, sl],
                    Alu.mult, Alu.mult,
                    accum_out=s12[:, ATT_TILES + t : ATT_TILES + t + 1],
                )

            def diff(k):
                # global chunk k -> tile t, chunk c within tile
                for c0, nch in MSE_TILING:
                    if c0 <= k < c0 + nch:
                        c = k - c0
                        base = 2 * c0 * MSE_CH
                        w = nch * MSE_CH
                        break
                xsl = slice(base + c * MSE_CH, base + (c + 1) * MSE_CH)
                gsl = slice(base + w + c * MSE_CH, base + w + (c + 1) * MSE_CH)
                jd = dpool.tile([128, MSE_CH], F16, tag="jd")
                nc.gpsimd.tensor_tensor(jd[:, :], xgt[:, gsl], xgt[:, xsl],
                                        Alu.subtract)
                return jd

            def sq(k, jd):
                nc.vector.scalar_tensor_tensor(
                    jd[:, :], jd[:, :], 1.0, jd[:, :],
                    Alu.mult, Alu.mult,
                    accum_out=fin[:, k : k + 1],
                )

            junk = spool.tile([128, 256], F32, tag="junk")
            psum_s = ppool.tile([1, 256], F32, tag="ps")

            def mem_d(i):
                msl = slice(i * 256, (i + 1) * 256)
                nc.vector.scalar_tensor_tensor(
                    junk[:, :], mt[:, msl], 1.0, mt[:, msl],
                    Alu.mult, Alu.mult,
                    accum_out=fin[:, 6 + i : 7 + i],
                )
                nc.tensor.matmul(
                    psum_s[:, :], ones[:, :], mt[:, msl],
                    start=(i == 0), stop=(i == 1),
                )

            xe(0); xe(1)
            j0 = diff(0); j1 = diff(1)
            sq(0, j0)
            xe(2)
            sq(1, j1)
            xe(3)
            j2 = diff(2); j3 = diff(3)
            sq(2, j2)
            xe(4)
            sq(3, j3)
            xe(5)
            j4 = diff(4); j5 = diff(5)
            sq(4, j4)
            xe(6)
            sq(5, j5)
            mem_d(0); mem_d(1)
            xe(7)

            # --- fold mse/mem partials and store o (completes while the
            # entropy tail still runs; rr right after the last x*e) ---
            psum_p = ppool.tile([1, NP], F32, tag="pp")
            nc.tensor.matmul(
                psum_p[:, :], ones[:, :], fin[:, :], start=True, stop=True
            )
            osb = spool.tile([1, NP + 256], F32, tag="osb")
            nc.vector.tensor_copy(osb[:, 0:NP], psum_p[:, :])
            nc.vector.tensor_copy(osb[:, NP:], psum_s[:, :])
            nc.gpsimd.dma_start(o_out[:, :], osb[:, :])
            nc.sync.dma_start(rr_out[:, :], s12[:, :])

          if loop_iters is not None and loop_iters > 1:
              with tc.For_i(0, loop_iters, 1):
                  body()
          else:
              body()

    nc.finalize()
    return nc


def _get_program():
    global _prog
    if _prog is None:
        _prog = _build_program()
    return _prog


def _make_in_maps(output, ground_truth, att, mem):
    o8 = np.asarray(output).reshape(-1).astype(FP8NP)
    g8 = np.asarray(ground_truth).reshape(-1).astype(FP8NP)
    a16 = np.asarray(att).astype(np.float16)
    mem64 = np.asarray(mem, dtype=np.float64)
    norms = np.maximum(np.linalg.norm(mem64, axis=1, keepdims=True), 1e-8)
    u = (mem64 / norms).astype(np.float32)
    per = MSE_N // N_CORES
    upad = np.zeros((256 - MEM_ROWS, 256), dtype=np.float32)
    in_maps = []
    for c in range(N_CORES):
        xc = o8[c * per : (c + 1) * per].reshape(128, MSE_FREE)
        gc = g8[c * per : (c + 1) * per].reshape(128, MSE_FREE)
        xgc = np.empty((128, 2 * MSE_FREE), dtype=FP8NP)
        off = 0
        for c0, nch in MSE_TILING:
            w = nch * MSE_CH
            xgc[:, off : off + w] = xc[:, c0 * MSE_CH : c0 * MSE_CH + w]
            xgc[:, off + w : off + 2 * w] = gc[:, c0 * MSE_CH : c0 * MSE_CH + w]
            off += 2 * w
        ac = a16[c * 1024 : (c + 1) * 1024].reshape(ATT_TILES, 128, ATT_F)
        apk = np.ascontiguousarray(ac.transpose(1, 0, 2)).reshape(128, ATT_TILES * ATT_F)
        us = np.concatenate([u[c * MEM_ROWS : (c + 1) * MEM_ROWS], upad]).reshape(2, 128, 256)
        mpk = np.concatenate([us[0], us[1]], axis=1)  # [128, 512]
        in_maps.append({"xg": xgc, "a": apk, "m": mpk})
    return in_maps


def _combine(results):
    o = np.stack([np.asarray(r["o"], np.float64).reshape(NP + 256) for r in results])
    p, s = o[:, :NP], o[:, NP:]
    ssd = p[:, 0:6].sum()
    d = p[:, 6:8].sum()
    sv = s.sum(axis=0)
    reg = 0.0
    for r in results:
        rr = np.asarray(r["rr"], np.float64).reshape(128, 2 * ATT_TILES)
        s1, s2 = rr[:, :ATT_TILES], rr[:, ATT_TILES:]
        reg += float((s2 / s1 - np.log(s1)).sum())
    loss = ssd / MSE_N - REG_PARAM * reg + 0.5 * (sv @ sv - d)
    return np.array(loss, dtype=np.float32)


def run(output, ground_truth, att, mem, **spmd_kwargs):
    nc = _get_program()
    in_maps = _make_in_maps(output, ground_truth, att, mem)
    res = run_bass_kernel_spmd(nc, in_maps, list(range(N_CORES)), **spmd_kwargs)
    return _combine(res.results), res


def kernel(output, ground_truth, att, mem):
    out, _ = run(output, ground_truth, att, mem)
    return out


# revision 13
# speedup vs baseline: 1.0078x; 1.0078x over previous
"""MemAELoss (MSE + entropy regularizer + pairwise-cosine memory penalty) on 8 trn2 cores.

Math (validated vs reference):
  loss = mean((g-o)^2) - 2e-4 * sum(softmax(att)*log_softmax(att))
         + sum_{i<j} cos(mem_i, mem_j)

Reformulations:
  * entropy per row, no max-subtraction needed (|att| < 6): S1 = sum e^x,
    S2 = sum x*e^x, row_term = S2/S1 - ln(S1). Per-row S1/S2 are exported
    and the tiny ln-finalize (8K rows) runs on the host during the gather,
    which keeps the ACT table set fixed (exp only) on device.
  * cosine triu sum: with u_i = mem_i/||mem_i||,
      sum_{i<j} u_i.u_j = 0.5*(||sum_i u_i||^2 - sum_i ||u_i||^2)
    so each core only produces a 256-vector s_c and a scalar d_c. The row
    normalization (u = mem/||mem||) happens on the host during input
    marshaling; the device computes the O(N^2)-equivalent reduction
    (s = ones^T @ u on PE, d = sum u^2 via DVE accum).

Sharding: pure data-parallel across 8 cores (output/ground_truth by flat
range, att by rows, mem by rows padded 250->256 with zero rows).

Performance structure (per core, ~16.5us DMA floor, ~16.3us ACT floor):
  * output/ground_truth ship as fp8 e3m4 (the loss is a statistical
    aggregate; e3m4 rounding perturbs it ~1.4e-5 rel, measured end-to-end,
    vs 2e-2 tolerance). att ships as f16: the x*e^x multiply must run on
    DVE in its 2x 16-bit mode (a 1-byte operand drops DVE to 1x), and no
    engine/DMA path upcasts fp8->f16 at line rate (the SWDGE cast DMA
    works but runs ~30x below line rate - measured 145us/iter). HBM per
    core: 5.93 MB vs 7.5 MB for the all-f16 version.
  * xg (packed [x_t | g_t] mse tiles) stays fp8 in SBUF: Pool subtracts
    fp8-fp8 -> f16 exactly; DVE squares the f16 diff at 2x with f32 accum.
  * engine placement: ACT does ONLY the 8 exps (f16 out + f32 S1 accum),
    ~2.0us each - the per-core compute bottleneck (~16.3us). DVE: 8 x*e
    STT at 2x (+S2 accum), 6 mse squares, 2 mem-d accums, PSUM copies
    (~13.5us). Pool: 6 mse diffs (~8.5us). PE: 3 tiny matmuls. SP: all
    HWDGE loads + 2 stores (FIFO per ring => transfers arrive in emission
    order at full bandwidth, no concurrent-DMA bandwidth splitting).
  * loads emitted in consumption order (att singles with xg/m interleaved,
    a7 last-but-early); compute emitted in data-arrival order with mse
    squares slotted into the DVE gaps between x*e ops. The mse/mem
    finalize + o store complete before the entropy stream ends; the rr
    (S1/S2) store right after the last x*e is the tail.
  * avoid tensor_tensor_reduce and DMA accum_op - both fault on this
    toolchain/HW. Multi-sem waits legalized by Bacc's event semaphores.
"""

import sys

sys.path.insert(0, "/opt/trn_rl_repo")

import ml_dtypes
import numpy as np

import concourse.bacc as bacc
import concourse.tile as tile
from concourse import mybir
from concourse.bass_utils import run_bass_kernel_spmd
from concourse.tile import add_dep_helper

F32 = mybir.dt.float32
F16 = mybir.dt.float16
F8 = mybir.dt.float8e3
FP8NP = ml_dtypes.float8_e3m4
Alu = mybir.AluOpType
Act = mybir.ActivationFunctionType

N_CORES = 8
MSE_N = 32 * 3 * 256 * 256  # 6291456 total elements
MSE_FREE = 6144             # per-core: 128 x 6144 (x and g each)
MSE_CH = 1024               # mse accum-chunk width (6 chunks)
ATT_TILES = 8               # per-core att rows 1024 = 8 x 128
ATT_F = 2000
MSE_TILING = [(0, 2), (2, 2), (4, 1), (5, 1)]          # (chunk0, nchunks)
MEM_ROWS = 250              # per-core mem rows, padded to 256 (2 x 128)
REG_PARAM = 2e-4
NP = 8                      # 6 mse ssd chunks, 2 d

_prog = None


def _build_program(loop_iters=None):
    nc = bacc.Bacc()
    # All big inputs are fp8 e3m4 on the wire. xg packs [x_t | g_t] blocks
    # so mse x/g slices are adjacent; a packs att row-groups so partition p
    # holds rows {p, 128+p, ...} concatenated (any [128, k*2000] slice is
    # one clean 2D DMA). m holds host-normalized unit rows (pad rows = 0).
    xg = nc.declare_dram_parameter("xg", [128, 2 * MSE_FREE], F8, isOutput=False)
    a = nc.declare_dram_parameter("a", [128, ATT_TILES * ATT_F], F16, isOutput=False)
    m = nc.declare_dram_parameter("m", [128, 512], F32, isOutput=False)
    o_out = nc.declare_dram_parameter("o", [1, NP + 256], F32, isOutput=True)
    rr_out = nc.declare_dram_parameter("rr", [128, 2 * ATT_TILES], F32, isOutput=True)

    with tile.TileContext(nc) as tc:
        with (
            tc.tile_pool(name="att_in", bufs=2) as apool,
            tc.tile_pool(name="att_exp", bufs=2) as epool,
            tc.tile_pool(name="mse_in", bufs=2) as xpool,
            tc.tile_pool(name="mse_diff", bufs=4) as dpool,
            tc.tile_pool(name="mem", bufs=2) as mpool,
            tc.tile_pool(name="stats", bufs=2) as spool,
            tc.tile_pool(name="psum", bufs=2, space="PSUM") as ppool,
        ):

          def body(_iv=None):
            s12 = spool.tile([128, 2 * ATT_TILES], F32, tag="s12")
            fin = spool.tile([128, NP], F32, tag="fin")
            ones = spool.tile([128, 1], F32, tag="ones")
            nc.vector.memset(ones[:, :], 1.0)

            at = apool.tile([128, ATT_TILES * ATT_F], F16, tag="at")
            et = epool.tile([128, ATT_TILES * ATT_F], F16, tag="et")
            xgt = xpool.tile([128, 2 * MSE_FREE], F8, tag="xg")
            mt = mpool.tile([128, 512], F32, tag="m")

            # --- loads. Measured DMA behavior here: HWDGE transfers run at
            # full rate but each dma_start pays ~2us of serialized
            # completion latency on its ring (15 DMAs/iter => 46us/iter);
            # SWDGE (gpsimd) moves data at only ~33 GB/s. So: merge the big
            # loads and split them across the two HWDGE rings (att on SP,
            # xg on ACT - one extra ACT dispatch ~0.7us), and put only the
            # tiny m load / o store on SWDGE, where Pool pays ~1us of
            # descriptor-gen and the slow transfer rides async off the
            # critical path. rr (the tail output) stays on SP. At steady
            # state (For_i, bufs=2) iteration k+1's loads overlap iteration
            # k's compute, so arrival granularity costs nothing.
            def a_load(t):
                sl = slice(t * ATT_F, (t + 1) * ATT_F)
                nc.sync.dma_start(at[:, sl], a[:, sl])

            def xg_load(t):
                c0, nch = MSE_TILING[t]
                w = nch * MSE_CH
                base = 2 * c0 * MSE_CH
                sl = slice(base, base + 2 * w)
                nc.sync.dma_start(xgt[:, sl], xg[:, sl])

            a_load(0); a_load(1); xg_load(0)
            a_load(2); a_load(3); xg_load(1)
            a_load(4); a_load(5); xg_load(2)
            a_load(6); a_load(7); xg_load(3)
            nc.sync.dma_start(mt[:, :], m[:, :])

            # --- ACT: the 8 exps, in arrival order (the only ACT work) ---
            for t in range(ATT_TILES):
                sl = slice(t * ATT_F, (t + 1) * ATT_F)
                nc.scalar.activation(
                    et[:, sl], at[:, sl], Act.Exp,
                    accum_out=s12[:, t : t + 1],
                )

            # --- DVE/Pool, emitted in expected arrival order; mse squares
            # and mem accums slot into the DVE idle gaps between x*e ops ---
            def xe(t):
                sl = slice(t * ATT_F, (t + 1) * ATT_F)
                nc.vector.scalar_tensor_tensor(
                    et[:, sl], at[:, sl], 1.0, et[:, sl],
                    Alu.mult, Alu.mult,
                    accum_out=s12[:, ATT_TILES + t : ATT_TILES + t + 1],
                )

            def diff(k):
                # global chunk k -> tile t, chunk c within tile
                for c0, nch in MSE_TILING:
                    if c0 <= k < c0 + nch:
                        c = k - c0
                        base = 2 * c0 * MSE_CH
                        w = nch * MSE_CH
                        break
                xsl = slice(base + c * MSE_CH, base + (c + 1) * MSE_CH)
                gsl = slice(base + w + c * MSE_CH, base + w + (c + 1) * MSE_CH)
                jd = dpool.tile([128, MSE_CH], F16, tag="jd")
                nc.gpsimd.tensor_tensor(jd[:, :], xgt[:, gsl], xgt[:, xsl],
                                        Alu.subtract)
                return jd

            def sq(k, jd):
                nc.vector.scalar_tensor_tensor(
                    jd[:, :], jd[:, :], 1.0, jd[:, :],
                    Alu.mult, Alu.mult,
                    accum_out=fin[:, k : k + 1],
                )

            junk = spool.tile([128, 256], F32, tag="junk")
            psum_s = ppool.tile([1, 256], F32, tag="ps")

            def mem_d(i):
                msl = slice(i * 256, (i + 1) * 256)
                nc.vector.scalar_tensor_tensor(
                    junk[:, :], mt[:, msl], 1.0, mt[:, msl],
                    Alu.mult, Alu.mult,
                    accum_out=fin[:, 6 + i : 7 + i],
                )
                nc.tensor.matmul(
                    psum_s[:, :], ones[:, :], mt[:, msl],
                    start=(i == 0), stop=(i == 1),
                )

            xe(0); xe(1)
            j0 = diff(0); j1 = diff(1)
            sq(0, j0)
            xe(2)
            sq(1, j1)
            xe(3)
            j2 = diff(2); j3 = diff(3)
            sq(2, j2)
            xe(4)
            sq(3, j3)
            xe(5)
            j4 = diff(4); j5 = diff(5)
            sq(4, j4)
            xe(6)
            sq(5, j5)
            mem_d(0); mem_d(1)
            xe(7)

            # --- fold mse/mem partials and store o (completes while the
            # entropy tail still runs; rr right after the last x*e) ---
            psum_p = ppool.tile([1, NP], F32, tag="pp")
            nc.tensor.matmul(
                psum_p[:, :], ones[:, :], fin[:, :], start=True, stop=True
            )
            osb = spool.tile([1, NP + 256], F32, tag="osb")
            nc.vector.tensor_copy(osb[:, 0:NP], psum_p[:, :])
            nc.vector.tensor_copy(osb[:, NP:], psum_s[:, :])
            nc.gpsimd.dma_start(o_out[:, :], osb[:, :])
            nc.gpsimd.dma_start(rr_out[:, :], s12[:, :])

          if loop_iters is not None and loop_iters > 1:
              with tc.For_i(0, loop_iters, 1):
                  body()
          else:
              body()

    nc.finalize()
    return nc


def _get_program():
    global _prog
    if _prog is None:
        _prog = _build_program()
    return _prog


def _make_in_maps(output, ground_truth, att, mem):
    o8 = np.asarray(output).reshape(-1).astype(FP8NP)
    g8 = np.asarray(ground_truth).reshape(-1).astype(FP8NP)
    a16 = np.asarray(att).astype(np.float16)
    mem64 = np.asarray(mem, dtype=np.float64)
    norms = np.maximum(np.linalg.norm(mem64, axis=1, keepdims=True), 1e-8)
    u = (mem64 / norms).astype(np.float32)
    per = MSE_N // N_CORES
    upad = np.zeros((256 - MEM_ROWS, 256), dtype=np.float32)
    in_maps = []
    for c in range(N_CORES):
        xc = o8[c * per : (c + 1) * per].reshape(128, MSE_FREE)
        gc = g8[c * per : (c + 1) * per].reshape(128, MSE_FREE)
        xgc = np.empty((128, 2 * MSE_FREE), dtype=FP8NP)
        off = 0
        for c0, nch in MSE_TILING:
            w = nch * MSE_CH
            xgc[:, off : off + w] = xc[:, c0 * MSE_CH : c0 * MSE_CH + w]
            xgc[:, off + w : off + 2 * w] = gc[:, c0 * MSE_CH : c0 * MSE_CH + w]
            off += 2 * w
        ac = a16[c * 1024 : (c + 1) * 1024].reshape(ATT_TILES, 128, ATT_F)
        apk = np.ascontiguousarray(ac.transpose(1, 0, 2)).reshape(128, ATT_TILES * ATT_F)
        us = np.concatenate([u[c * MEM_ROWS : (c + 1) * MEM_ROWS], upad]).reshape(2, 128, 256)
        mpk = np.concatenate([us[0], us[1]], axis=1)  # [128, 512]
        in_maps.append({"xg": xgc, "a": apk, "m": mpk})
    return in_maps


def _combine(results):
    o = np.stack([np.asarray(r["o"], np.float64).reshape(NP + 256) for r in results])
    p, s = o[:, :NP], o[:, NP:]
    ssd = p[:, 0:6].sum()
    d = p[:, 6:8].sum()
    sv = s.sum(axis=0)
    reg = 0.0
    for r in results:
        rr = np.asarray(r["rr"], np.float64).reshape(128, 2 * ATT_TILES)
        s1, s2 = rr[:, :ATT_TILES], rr[:, ATT_TILES:]
        reg += float((s2 / s1 - np.log(s1)).sum())
    loss = ssd / MSE_N - REG_PARAM * reg + 0.5 * (sv @ sv - d)
    return np.array(loss, dtype=np.float32)


def run(output, ground_truth, att, mem, **spmd_kwargs):
    nc = _get_program()
    in_maps = _make_in_maps(output, ground_truth, att, mem)
    res = run_bass_kernel_spmd(nc, in_maps, list(range(N_CORES)), **spmd_kwargs)
    return _combine(res.results), res


def kernel(output, ground_truth, att, mem):
    out, _ = run(output, ground_truth, att, mem)
    return out


# revision 14
# speedup vs baseline: 1.4121x; 1.4012x over previous
"""MemAELoss (MSE + entropy regularizer + pairwise-cosine memory penalty) on 8 trn2 cores.

Math (validated vs reference):
  loss = mean((g-o)^2) - 2e-4 * sum(softmax(att)*log_softmax(att))
         + sum_{i<j} cos(mem_i, mem_j)

Reformulations:
  * entropy per row, no max-subtraction needed (|att| < 6): S1 = sum e^x,
    S2 = sum x*e^x, row_term = S2/S1 - ln(S1). Per-row S1/S2 are exported
    and the tiny ln-finalize (8K rows) runs on the host during the gather,
    which keeps the ACT table set fixed (exp only) on device.
  * cosine triu sum: with u_i = mem_i/||mem_i||,
      sum_{i<j} u_i.u_j = 0.5*(||sum_i u_i||^2 - sum_i ||u_i||^2)
    so each core only produces a 256-vector s_c and a scalar d_c. The row
    normalization (u = mem/||mem||) happens on the host during input
    marshaling; the device computes the O(N^2)-equivalent reduction
    (s = ones^T @ u on PE, d = sum u^2 via DVE accum).

Sharding: pure data-parallel across 8 cores (output/ground_truth by flat
range, att by rows, mem by rows padded 250->256 with zero rows).

Performance structure (per core, ~16.5us DMA floor, ~16.3us ACT floor):
  * output/ground_truth ship as fp8 e3m4 (the loss is a statistical
    aggregate; e3m4 rounding perturbs it ~1.4e-5 rel, measured end-to-end,
    vs 2e-2 tolerance). att ships as f16: the x*e^x multiply must run on
    DVE in its 2x 16-bit mode (a 1-byte operand drops DVE to 1x), and no
    engine/DMA path upcasts fp8->f16 at line rate (the SWDGE cast DMA
    works but runs ~30x below line rate - measured 145us/iter). HBM per
    core: 5.93 MB vs 7.5 MB for the all-f16 version.
  * xg (packed [x_t | g_t] mse tiles) stays fp8 in SBUF: Pool subtracts
    fp8-fp8 -> f16 exactly; DVE squares the f16 diff at 2x with f32 accum.
  * engine placement: ACT does ONLY the 8 exps (f16 out + f32 S1 accum),
    ~2.0us each - the per-core compute bottleneck (~16.3us). DVE: 8 x*e
    STT at 2x (+S2 accum), 6 mse squares, 2 mem-d accums, PSUM copies
    (~13.5us). Pool: 6 mse diffs (~8.5us). PE: 3 tiny matmuls. SP: all
    HWDGE loads + 2 stores (FIFO per ring => transfers arrive in emission
    order at full bandwidth, no concurrent-DMA bandwidth splitting).
  * loads emitted in consumption order (att singles with xg/m interleaved,
    a7 last-but-early); compute emitted in data-arrival order with mse
    squares slotted into the DVE gaps between x*e ops. The mse/mem
    finalize + o store complete before the entropy stream ends; the rr
    (S1/S2) store right after the last x*e is the tail.
  * avoid tensor_tensor_reduce and DMA accum_op - both fault on this
    toolchain/HW. Multi-sem waits legalized by Bacc's event semaphores.
"""

import sys

sys.path.insert(0, "/opt/trn_rl_repo")

import ml_dtypes
import numpy as np

import concourse.bacc as bacc
import concourse.tile as tile
from concourse import mybir
from concourse.bass_utils import run_bass_kernel_spmd
from concourse.tile import add_dep_helper

F32 = mybir.dt.float32
F16 = mybir.dt.float16
F8 = mybir.dt.float8e3
FP8NP = ml_dtypes.float8_e3m4
Alu = mybir.AluOpType
Act = mybir.ActivationFunctionType

N_CORES = 8
MSE_N = 32 * 3 * 256 * 256  # 6291456 total elements
MSE_FREE = 6144             # per-core: 128 x 6144 (x and g each)
MSE_CH = 1024               # mse accum-chunk width (6 chunks)
ATT_TILES = 8               # per-core att rows 1024 = 8 x 128
ATT_F = 2000
MSE_TILING = [(0, 2), (2, 2), (4, 1), (5, 1)]          # (chunk0, nchunks)
MEM_ROWS = 250              # per-core mem rows, padded to 256 (2 x 128)
REG_PARAM = 2e-4
NP = 8                      # 6 mse ssd chunks, 2 d

_prog = None


def _build_program(loop_iters=None):
    nc = bacc.Bacc()
    # All big inputs are fp8 e3m4 on the wire. xg packs [x_t | g_t] blocks
    # so mse x/g slices are adjacent; a packs att row-groups so partition p
    # holds rows {p, 128+p, ...} concatenated (any [128, k*2000] slice is
    # one clean 2D DMA). m holds host-normalized unit rows (pad rows = 0).
    xg = nc.declare_dram_parameter("xg", [128, 2 * MSE_FREE], F8, isOutput=False)
    a = nc.declare_dram_parameter("a", [128, ATT_TILES * ATT_F], F8, isOutput=False)
    m = nc.declare_dram_parameter("m", [128, 512], F32, isOutput=False)
    o_out = nc.declare_dram_parameter("o", [1, NP + 256], F32, isOutput=True)
    rr_out = nc.declare_dram_parameter("rr", [128, 2 * ATT_TILES], F32, isOutput=True)

    with tile.TileContext(nc) as tc:
        with (
            tc.tile_pool(name="att_in", bufs=2) as apool,
            tc.tile_pool(name="att_exp", bufs=2) as epool,
            tc.tile_pool(name="mse_in", bufs=2) as xpool,
            tc.tile_pool(name="mse_diff", bufs=4) as dpool,
            tc.tile_pool(name="mem", bufs=2) as mpool,
            tc.tile_pool(name="stats", bufs=2) as spool,
            tc.tile_pool(name="psum", bufs=2, space="PSUM") as ppool,
        ):

          def body(_iv=None):
            s12 = spool.tile([128, 2 * ATT_TILES], F32, tag="s12")
            fin = spool.tile([128, NP], F32, tag="fin")
            ones = spool.tile([128, 1], F32, tag="ones")
            nc.vector.memset(ones[:, :], 1.0)

            at = apool.tile([128, ATT_TILES * ATT_F], F8, tag="at")
            et = epool.tile([128, ATT_TILES * ATT_F], F16, tag="et")
            xgt = xpool.tile([128, 2 * MSE_FREE], F8, tag="xg")
            mt = mpool.tile([128, 512], F32, tag="m")

            # --- loads, all HWDGE on the SP ring (FIFO => sequential
            # arrival in emission order at full bandwidth). Interleaved so
            # every consumer's data lands just before its engine slot:
            # a0 a1 xg0 a2 a3 xg1 a4 a5 xg2 a6 a7 xg3 m
            def a_load(t):
                sl = slice(t * ATT_F, (t + 1) * ATT_F)
                nc.sync.dma_start(at[:, sl], a[:, sl])

            def xg_load(t):
                c0, nch = MSE_TILING[t]
                w = nch * MSE_CH
                base = 2 * c0 * MSE_CH
                sl = slice(base, base + 2 * w)
                nc.sync.dma_start(xgt[:, sl], xg[:, sl])

            a_load(0); a_load(1); xg_load(0)
            a_load(2); a_load(3); xg_load(1)
            a_load(4); a_load(5); xg_load(2)
            a_load(6); a_load(7); xg_load(3)
            nc.sync.dma_start(mt[:, :], m[:, :])

            # --- ACT: the 8 exps, in arrival order (the only ACT work) ---
            for t in range(ATT_TILES):
                sl = slice(t * ATT_F, (t + 1) * ATT_F)
                nc.scalar.activation(
                    et[:, sl], at[:, sl], Act.Exp,
                    accum_out=s12[:, t : t + 1],
                )

            # --- DVE/Pool, emitted in expected arrival order; mse squares
            # and mem accums slot into the DVE idle gaps between x*e ops ---
            def xe(t):
                sl = slice(t * ATT_F, (t + 1) * ATT_F)
                nc.vector.scalar_tensor_tensor(
                    et[:, sl], at[:, sl], 1.0, et[:, sl],
                    Alu.mult, Alu.mult,
                    accum_out=s12[:, ATT_TILES + t : ATT_TILES + t + 1],
                )

            def diff(k):
                # global chunk k -> tile t, chunk c within tile
                for c0, nch in MSE_TILING:
                    if c0 <= k < c0 + nch:
                        c = k - c0
                        base = 2 * c0 * MSE_CH
                        w = nch * MSE_CH
                        break
                xsl = slice(base + c * MSE_CH, base + (c + 1) * MSE_CH)
                gsl = slice(base + w + c * MSE_CH, base + w + (c + 1) * MSE_CH)
                jd = dpool.tile([128, MSE_CH], F16, tag="jd")
                nc.gpsimd.tensor_tensor(jd[:, :], xgt[:, gsl], xgt[:, xsl],
                                        Alu.subtract)
                return jd

            def sq(k, jd):
                nc.vector.scalar_tensor_tensor(
                    jd[:, :], jd[:, :], 1.0, jd[:, :],
                    Alu.mult, Alu.mult,
                    accum_out=fin[:, k : k + 1],
                )

            junk = spool.tile([128, 256], F32, tag="junk")
            psum_s = ppool.tile([1, 256], F32, tag="ps")

            def mem_d(i):
                msl = slice(i * 256, (i + 1) * 256)
                nc.vector.scalar_tensor_tensor(
                    junk[:, :], mt[:, msl], 1.0, mt[:, msl],
                    Alu.mult, Alu.mult,
                    accum_out=fin[:, 6 + i : 7 + i],
                )
                nc.tensor.matmul(
                    psum_s[:, :], ones[:, :], mt[:, msl],
                    start=(i == 0), stop=(i == 1),
                )

            xe(0); xe(1)
            j0 = diff(0); j1 = diff(1)
            sq(0, j0)
            xe(2)
            sq(1, j1)
            xe(3)
            j2 = diff(2); j3 = diff(3)
            sq(2, j2)
            xe(4)
            sq(3, j3)
            xe(5)
            j4 = diff(4); j5 = diff(5)
            sq(4, j4)
            xe(6)
            sq(5, j5)
            mem_d(0); mem_d(1)
            xe(7)

            # --- fold mse/mem partials and store o (completes while the
            # entropy tail still runs; rr right after the last x*e) ---
            psum_p = ppool.tile([1, NP], F32, tag="pp")
            nc.tensor.matmul(
                psum_p[:, :], ones[:, :], fin[:, :], start=True, stop=True
            )
            osb = spool.tile([1, NP + 256], F32, tag="osb")
            nc.vector.tensor_copy(osb[:, 0:NP], psum_p[:, :])
            nc.vector.tensor_copy(osb[:, NP:], psum_s[:, :])
            nc.sync.dma_start(rr_out[:, :], s12[:, :])
            nc.sync.dma_start(o_out[:, :], osb[:, :])

          if loop_iters is not None and loop_iters > 1:
              with tc.For_i(0, loop_iters, 1):
                  body()
          else:
              body()

    nc.finalize()
    return nc


def _get_program():
    global _prog
    if _prog is None:
        _prog = _build_program()
    return _prog


def _make_in_maps(output, ground_truth, att, mem):
    o8 = np.asarray(output).reshape(-1).astype(FP8NP)
    g8 = np.asarray(ground_truth).reshape(-1).astype(FP8NP)
    a16 = np.asarray(att).astype(FP8NP)
    mem64 = np.asarray(mem, dtype=np.float64)
    norms = np.maximum(np.linalg.norm(mem64, axis=1, keepdims=True), 1e-8)
    u = (mem64 / norms).astype(np.float32)
    per = MSE_N // N_CORES
    upad = np.zeros((256 - MEM_ROWS, 256), dtype=np.float32)
    in_maps = []
    for c in range(N_CORES):
        xc = o8[c * per : (c + 1) * per].reshape(128, MSE_FREE)
        gc = g8[c * per : (c + 1) * per].reshape(128, MSE_FREE)
        xgc = np.empty((128, 2 * MSE_FREE), dtype=FP8NP)
        off = 0
        for c0, nch in MSE_TILING:
            w = nch * MSE_CH
            xgc[:, off : off + w] = xc[:, c0 * MSE_CH : c0 * MSE_CH + w]
            xgc[:, off + w : off + 2 * w] = gc[:, c0 * MSE_CH : c0 * MSE_CH + w]
            off += 2 * w
        ac = a16[c * 1024 : (c + 1) * 1024].reshape(ATT_TILES, 128, ATT_F)
        apk = np.ascontiguousarray(ac.transpose(1, 0, 2)).reshape(128, ATT_TILES * ATT_F)
        us = np.concatenate([u[c * MEM_ROWS : (c + 1) * MEM_ROWS], upad]).reshape(2, 128, 256)
        mpk = np.concatenate([us[0], us[1]], axis=1)  # [128, 512]
        in_maps.append({"xg": xgc, "a": apk, "m": mpk})
    return in_maps


def _combine(results):
    o = np.stack([np.asarray(r["o"], np.float64).reshape(NP + 256) for r in results])
    p, s = o[:, :NP], o[:, NP:]
    ssd = p[:, 0:6].sum()
    d = p[:, 6:8].sum()
    sv = s.sum(axis=0)
    reg = 0.0
    for r in results:
        rr = np.asarray(r["rr"], np.float64).reshape(128, 2 * ATT_TILES)
        s1, s2 = rr[:, :ATT_TILES], rr[:, ATT_TILES:]
        reg += float((s2 / s1 - np.log(s1)).sum())
    loss = ssd / MSE_N - REG_PARAM * reg + 0.5 * (sv @ sv - d)
    return np.array(loss, dtype=np.float32)


def run(output, ground_truth, att, mem, **spmd_kwargs):
    nc = _get_program()
    in_maps = _make_in_maps(output, ground_truth, att, mem)
    res = run_bass_kernel_spmd(nc, in_maps, list(range(N_CORES)), **spmd_kwargs)
    return _combine(res.results), res


def kernel(output, ground_truth, att, mem):
    out, _ = run(output, ground_truth, att, mem)
    return out
